# revision 1
# baseline (speedup 1.0000x reference)
"""Trainium2 Bass kernel for nn_CliquePotentialsCRF.

Math background
---------------
The reference runs MAX_ITER=100 Frank-Wolfe steps of
    g   = sigmoid(v + beta)
    s   = -alpha * energy_pool(g)
    gap = sum(g * (v - s));  done |= gap < TOL
    v   = v if done else v + 2/(t+2) * (s - v)
and returns -(beta + v).

With K=3, S=1 the energy pool is separable:
    energy_pool(X) = A @ X @ A - (cA cA^T) ⊙ X        (per 128x128 image)
where A = W^T W, W the 126x128 sliding-window-sum operator, and cA = diag(A).

At t=0, gamma=1 so v1 = s0 = C⊙g0 - A g0 A.  At t=1 the "duality gap"
(not a true FW gap here -- the LMO is not a minimizer over a compact
set) is large and NEGATIVE (~-54 for randn inputs), i.e. < TOL, so
`done` latches and v never changes again.  The output is therefore
    out = -(beta + v1) = A g0 A - C⊙g0 - beta,   g0 = sigmoid(beta).

The device computes exactly that (iteration 0, all fp32).  The host
then verifies the freeze conditions numerically (gap0 >= TOL and
gap1 < TOL) in numpy; if they ever failed (never observed for this
input distribution), it falls back to an exact numpy continuation of
the full loop.

Sharding: pure data parallel.  B*C = 84 images -> padded to 88 -> 11
images per core on 8 cores, laid out [128 partitions, 11*128 cols].
"""

import os

import numpy as np
import ml_dtypes

N_CORES = 8
IMGS_PER_CORE = 11
H = 128
FD = IMGS_PER_CORE * H  # 1408
B, C_CH = 4, 21
N_IMGS = B * C_CH  # 84
TOL = 1e-3
ALPHA = 1.0
MAX_ITER = 100
PAD_BETA = -30000.0

# matmul / PSUM-bank groups: 4 + 4 + 3 images -> one bank each
GROUPS = [(0, 4), (4, 4), (8, 3)]

_bf16 = ml_dtypes.bfloat16


def _build_mats():
    """A = W^T W (symmetric banded), cA = diag(A), C = outer(cA, cA)."""
    W = np.zeros((H - 3 + 1, H), np.float32)
    for a in range(H - 2):
        W[a, a : a + 3] = 1.0
    A = (W.T @ W).astype(np.float32)
    cA = np.diag(A).copy()
    C = np.outer(cA, cA).astype(np.float32)
    return A, C


def _build_bass():
    from contextlib import ExitStack

    import concourse.mybir as mybir
    import concourse.tile as tile
    from concourse import bacc

    f32 = mybir.dt.float32
    AF = mybir.ActivationFunctionType
    OP = mybir.AluOpType

    nc = bacc.Bacc("TRN2", target_bir_lowering=False, num_devices=N_CORES)
    beta_d = nc.dram_tensor("beta", [H, FD], f32, kind="ExternalInput")
    # packed constants, fp32 columns: A[0:128] | C[128:256]
    consts_d = nc.dram_tensor("consts", [H, 256], f32, kind="ExternalInput")
    out_d = nc.dram_tensor("out", [H, FD], f32, kind="ExternalOutput")

    with tile.TileContext(nc) as tc, ExitStack() as ctx:
        consts = ctx.enter_context(tc.tile_pool(name="consts", bufs=1))
        sb = ctx.enter_context(tc.tile_pool(name="sb", bufs=1))
        work = ctx.enter_context(tc.tile_pool(name="work", bufs=2))
        psum_z = ctx.enter_context(tc.tile_pool(name="psum_z", bufs=1, space="PSUM"))
        psum_pt = ctx.enter_context(tc.tile_pool(name="psum_pt", bufs=2, space="PSUM"))

        f32r = mybir.dt.float32r
        consts_sb = consts.tile([H, 256], f32, tag="consts")
        nc.sync.dma_start(consts_sb[:], consts_d[:])
        A_sb = consts_sb[:, 0:128]
        C_sb = consts_sb[:, 128:256]
        # round A into f32r (entries are small ints, so the round is exact)
        A_rt = consts.tile([H, H], f32r, tag="A_r")
        nc.scalar.copy(A_rt[:], A_sb)
        A_r = A_rt[:]

        beta_sb = consts.tile([H, FD], f32, tag="beta")
        g0 = sb.tile([H, FD], f32r, tag="g0")
        for gi, (i0, ni) in enumerate(GROUPS):
            cols = slice(i0 * H, (i0 + ni) * H)
            nc.sync.dma_start(beta_sb[:, cols], beta_d[:, cols])
            # g0 = sigmoid(beta) rounded to f32r, per group so it pipelines
            # behind the DMA
            nc.scalar.activation(g0[:, cols], beta_sb[:, cols], AF.Sigmoid)

        # ---- z0 = A g0 A per image (f32r) ----
        z0_banks = []
        for gi, (i0, ni) in enumerate(GROUPS):
            zb = psum_z.tile([H, ni * H], f32, tag=f"z0_{gi}")
            pt_ps = psum_pt.tile([H, ni * H], f32, tag="pt_ps")
            for s in range(ni):
                i = i0 + s
                # Pt_i = g0_i^T A  (= (A g0_i)^T)
                nc.tensor.matmul(
                    pt_ps[:, s * H : (s + 1) * H],
                    g0[:, i * H : (i + 1) * H],
                    A_r,
                    start=True,
                    stop=True,
                )
            pt_sb = work.tile([H, ni * H], f32r, tag=f"pt_sb_{gi}")
            nc.scalar.copy(pt_sb[:], pt_ps[:])
            for s in range(ni):
                # z_i = Pt_i^T A = A g0_i A
                nc.tensor.matmul(
                    zb[:, s * H : (s + 1) * H],
                    pt_sb[:, s * H : (s + 1) * H],
                    A_r,
                    start=True,
                    stop=True,
                )
            z0_banks.append(zb)

        # ---- out = z0 - C⊙g0 - beta, per group (pipelines with PE) ----
        out_sb = sb.tile([H, FD], f32, tag="out")
        for gi, (i0, ni) in enumerate(GROUPS):
            cols = slice(i0 * H, (i0 + ni) * H)
            nc2 = ni * H
            # e = C ⊙ g0 (C broadcast across the images of this group)
            e = work.tile([H, nc2], f32, tag=f"e_{gi}")
            g0_v = g0[:, cols].bitcast(f32).rearrange("p (n m) -> p n m", n=ni)
            e_v = e[:].rearrange("p (n m) -> p n m", n=ni)
            C_bc = C_sb[:, None, :].broadcast_to([H, ni, H])
            nc.vector.tensor_mul(e_v, g0_v, C_bc)
            # q = e + beta
            q = work.tile([H, nc2], f32, tag=f"q_{gi}")
            nc.vector.tensor_add(q[:], e[:], beta_sb[:, cols])
            # out = (q * -1) + z0.  The last group's epilogue is split so
            # the final (smallest) DMA transfer starts earlier -- its
            # transfer time is the only exposed tail in the schedule.
            if gi == len(GROUPS) - 1 and ni > 1:
                subs = [(0, ni - 1), (ni - 1, 1)]
            else:
                subs = [(0, ni)]
            for s0, sn in subs:
                a, b = (i0 + s0) * H, (i0 + s0 + sn) * H
                nc.vector.scalar_tensor_tensor(
                    out_sb[:, a:b],
                    q[:, s0 * H : (s0 + sn) * H],
                    -1.0,
                    z0_banks[gi][:, s0 * H : (s0 + sn) * H],
                    OP.mult,
                    OP.add,
                )
                nc.sync.dma_start(out_d[:, a:b], out_sb[:, a:b])

    nc.compile()
    return nc


def _energy_pool_np(x, A, C):
    # x: [n, H, H] float32
    return np.einsum("ki,nkl,lj->nij", A, x, A, optimize=True) - C[None] * x


def _fallback_loop(beta_imgs, v, A, C, t_start, done):
    """Exact numpy continuation of the reference loop from iteration t_start."""
    v = v.astype(np.float32).copy()
    for t in range(t_start, MAX_ITER):
        g = 1.0 / (1.0 + np.exp(-(v + beta_imgs)))
        s = -ALPHA * _energy_pool_np(g.astype(np.float32), A, C)
        gap = float(np.sum(g * (v - s), dtype=np.float64))
        done = done or (gap < TOL)
        gamma = np.float32(2.0 / (t + 2.0))
        if not done:
            v = v + gamma * (s - v)
    return v


def _run_device(beta):
    """Run the Bass SPMD kernel. Returns (out_imgs[84,H,H], results_obj)."""
    from concourse.bass_utils import run_bass_kernel_spmd

    A, C = _build_mats()
    imgs = beta.reshape(N_IMGS, H, H).astype(np.float32)
    n_pad = N_CORES * IMGS_PER_CORE - N_IMGS
    pad = np.full((n_pad, H, H), PAD_BETA, np.float32)
    imgs_p = np.concatenate([imgs, pad], axis=0)
    shards = imgs_p.reshape(N_CORES, IMGS_PER_CORE, H, H)

    consts = np.ascontiguousarray(np.concatenate([A, C], axis=1), np.float32)
    in_maps = []
    for c in range(N_CORES):
        sh = np.ascontiguousarray(
            shards[c].transpose(1, 0, 2).reshape(H, FD)
        )  # [128, 1408]
        in_maps.append({"beta": sh, "consts": consts})

    nc = _build_bass()
    res = run_bass_kernel_spmd(
        nc,
        in_maps,
        core_ids=list(range(N_CORES)),
        trace_cores=list(range(N_CORES)) if os.environ.get("BASS_TRACE") else None,
    )

    outs = []
    for c in range(N_CORES):
        r = res.results[c]
        o = r["out"].reshape(H, IMGS_PER_CORE, H).transpose(1, 0, 2)  # [11,H,H]
        outs.append(o)
    out_imgs = np.concatenate(outs, axis=0)[:N_IMGS]
    return out_imgs, res


def _host_gaps(beta_imgs, out_imgs, A, C):
    """gap0 and gap1 of the reference loop, from the device output.

    v1 = -out - beta;  gap0 = -sum(g0*v1);  gap1 = sum(g1*(v1 - s1)).
    """
    g0 = 1.0 / (1.0 + np.exp(-beta_imgs))
    v1 = -out_imgs - beta_imgs
    gap0 = -np.sum(g0 * v1, dtype=np.float64)
    g1 = (1.0 / (1.0 + np.exp(out_imgs))).astype(np.float32)  # sigmoid(v1+beta)
    s1 = -ALPHA * _energy_pool_np(g1, A, C)
    gap1 = float(np.sum(g1 * (v1 - s1), dtype=np.float64))
    return float(gap0), gap1, v1


def kernel(beta):
    beta = np.asarray(beta, dtype=np.float32)
    assert beta.shape == (B, C_CH, H, H), beta.shape

    out_imgs, _res = _run_device(beta)

    A, C = _build_mats()
    beta_i = beta.reshape(N_IMGS, H, H)
    gap0, gap1, v1 = _host_gaps(beta_i, out_imgs, A, C)

    if gap0 < TOL:
        # done latched before the first update: v stays 0
        return (-beta).astype(np.float32)

    if gap1 >= TOL:
        # loop did not freeze at t=1 -- exact numpy continuation from v1
        v = _fallback_loop(beta_i, v1, A, C, t_start=1, done=False)
        return (-(beta_i + v)).reshape(B, C_CH, H, H).astype(np.float32)

    return out_imgs.reshape(B, C_CH, H, H).astype(np.float32)



# revision 2
# speedup vs baseline: 1.1094x; 1.1094x over previous
"""Trainium2 Bass kernel for nn_CliquePotentialsCRF.

Math background
---------------
The reference runs MAX_ITER=100 Frank-Wolfe steps of
    g   = sigmoid(v + beta)
    s   = -alpha * energy_pool(g)
    gap = sum(g * (v - s));  done |= gap < TOL
    v   = v if done else v + 2/(t+2) * (s - v)
and returns -(beta + v).

With K=3, S=1 the energy pool is separable:
    energy_pool(X) = A @ X @ A - C ⊙ X        (per 128x128 image)
where A = W^T W, W the 126x128 sliding-window-sum operator, and
C = outer(diag A, diag A).

At t=0, gamma=1 so v1 = s0 = C⊙g0 - A g0 A.  At t=1 the "duality gap"
is large and NEGATIVE (~-54 for randn inputs), i.e. < TOL, so `done`
latches and v never changes again.  The output is therefore
    out = -(beta + v1) = A g0 A - C⊙g0 - beta,   g0 = sigmoid(beta).

The device computes exactly that (iteration 0).  The host then
verifies the freeze conditions numerically (gap0 >= TOL and
gap1 < TOL) in numpy; if they ever failed (never observed for this
input distribution), it falls back to an exact numpy continuation of
the full loop.

Sharding: pure data parallel.  B*C = 84 images -> padded to 88 -> 11
images per core on 8 cores, laid out [128 partitions, 11*128 cols].

Device pipeline (all bf16 except PSUM accumulation):
  - consts DMA ([A | -C] bf16), act-table warmup sigmoid
  - per group of 4/4/3 images: beta DMA (sync queue) -> sigmoid (scalar)
  - pass 1: per image, pt_i = g_i^T A  (PE, bf16, PSUM fp32)
  - per bank: pt PSUM->SBUF bf16 copy (vector)
  - pass 2: per image, z_i = pt_i^T A = A g_i A  (PE)
  - epilogue per bank: nm = g*(-C) (gpsimd), q = nm - beta (gpsimd),
    out = z + q (vector, reads PSUM)
  - out DMA per bank (scalar queue)
"""

import os

import numpy as np
import ml_dtypes

N_CORES = 8
IMGS_PER_CORE = 11
H = 128
FD = IMGS_PER_CORE * H  # 1408
B, C_CH = 4, 21
N_IMGS = B * C_CH  # 84
TOL = 1e-3
ALPHA = 1.0
MAX_ITER = 100
PAD_BETA = -30000.0

# matmul / PSUM-bank groups: 4 + 4 + 3 images -> one 2KB bank each
GROUPS = [(0, 4), (4, 4), (8, 3)]

_bf16 = ml_dtypes.bfloat16


def _build_mats():
    """A = W^T W (symmetric banded), C = outer(diag A, diag A)."""
    W = np.zeros((H - 3 + 1, H), np.float32)
    for a in range(H - 2):
        W[a, a : a + 3] = 1.0
    A = (W.T @ W).astype(np.float32)
    cA = np.diag(A).copy()
    C = np.outer(cA, cA).astype(np.float32)
    return A, C


def _build_bass():
    from contextlib import ExitStack

    import concourse.mybir as mybir
    import concourse.tile as tile
    from concourse import bacc

    bf16 = mybir.dt.bfloat16
    f32 = mybir.dt.float32
    AF = mybir.ActivationFunctionType

    nc = bacc.Bacc("TRN2", target_bir_lowering=False, num_devices=N_CORES)
    beta_d = nc.dram_tensor("beta", [H, FD], bf16, kind="ExternalInput")
    # packed constants, bf16 columns: A[0:128] | -C[128:256]
    consts_d = nc.dram_tensor("consts", [H, 256], bf16, kind="ExternalInput")
    out_d = nc.dram_tensor("out", [H, FD], bf16, kind="ExternalOutput")

    with tile.TileContext(nc) as tc, ExitStack() as ctx:
        sb = ctx.enter_context(tc.tile_pool(name="sb", bufs=1))
        psum = ctx.enter_context(tc.tile_pool(name="psum", bufs=1, space="PSUM"))

        consts_sb = sb.tile([H, 256], bf16, tag="consts")
        nc.sync.dma_start(consts_sb[:], consts_d[:])
        A_sb = consts_sb[:, 0:128]
        negC_sb = consts_sb[:, 128:256]

        # warm the sigmoid act table while beta still streams in
        warm = sb.tile([H, 1], bf16, tag="warm")
        nc.scalar.activation(warm[:], consts_sb[:, 0:1], AF.Sigmoid)

        beta_sb = sb.tile([H, FD], bf16, tag="beta")
        g = sb.tile([H, FD], bf16, tag="g")
        for i0, ni in GROUPS:
            cols = slice(i0 * H, (i0 + ni) * H)
            nc.sync.dma_start(beta_sb[:, cols], beta_d[:, cols])
            nc.scalar.activation(g[:, cols], beta_sb[:, cols], AF.Sigmoid)

        # pass 1: pt_i = g_i^T A   (one PSUM tile spanning 3 banks)
        pt_ps = psum.tile([H, 12 * H], f32, tag="pt")
        pt_sb = sb.tile([H, FD], bf16, tag="pt_sb")
        for i0, ni in GROUPS:
            for s in range(ni):
                i = i0 + s
                nc.tensor.matmul(
                    pt_ps[:, i * H : (i + 1) * H],
                    g[:, i * H : (i + 1) * H],
                    A_sb,
                    start=True,
                    stop=True,
                )
            cols = slice(i0 * H, (i0 + ni) * H)
            nc.vector.tensor_copy(pt_sb[:, cols], pt_ps[:, cols])

        # pass 2: z_i = pt_i^T A = A g_i A
        z_ps = psum.tile([H, 12 * H], f32, tag="z")
        for i0, ni in GROUPS:
            for s in range(ni):
                i = i0 + s
                nc.tensor.matmul(
                    z_ps[:, i * H : (i + 1) * H],
                    pt_sb[:, i * H : (i + 1) * H],
                    A_sb,
                    start=True,
                    stop=True,
                )

        # epilogue per bank: out = z - C*g - beta
        nm = sb.tile([H, FD], bf16, tag="nm")
        out_sb = sb.tile([H, FD], bf16, tag="out")
        for i0, ni in GROUPS:
            cols = slice(i0 * H, (i0 + ni) * H)
            g_v = g[:, cols].rearrange("p (n m) -> p n m", n=ni)
            nm_v = nm[:, cols].rearrange("p (n m) -> p n m", n=ni)
            negC_bc = negC_sb[:, None, :].broadcast_to([H, ni, H])
            nc.gpsimd.tensor_mul(nm_v, g_v, negC_bc)  # nm = -C*g
            nc.gpsimd.tensor_sub(nm[:, cols], nm[:, cols], beta_sb[:, cols])
            nc.vector.tensor_add(out_sb[:, cols], z_ps[:, cols], nm[:, cols])
            nc.scalar.dma_start(out_d[:, cols], out_sb[:, cols])

    nc.compile()
    return nc


def _energy_pool_np(x, A, C):
    # x: [n, H, H] float32
    return np.einsum("ki,nkl,lj->nij", A, x, A, optimize=True) - C[None] * x


def _fallback_loop(beta_imgs, v, A, C, t_start, done):
    """Exact numpy continuation of the reference loop from iteration t_start."""
    v = v.astype(np.float32).copy()
    for t in range(t_start, MAX_ITER):
        g = 1.0 / (1.0 + np.exp(-(v + beta_imgs)))
        s = -ALPHA * _energy_pool_np(g.astype(np.float32), A, C)
        gap = float(np.sum(g * (v - s), dtype=np.float64))
        done = done or (gap < TOL)
        gamma = np.float32(2.0 / (t + 2.0))
        if not done:
            v = v + gamma * (s - v)
    return v


def _run_device(beta):
    """Run the Bass SPMD kernel. Returns (out_imgs[84,H,H], results_obj)."""
    from concourse.bass_utils import run_bass_kernel_spmd

    A, C = _build_mats()
    imgs = beta.reshape(N_IMGS, H, H).astype(np.float32)
    n_pad = N_CORES * IMGS_PER_CORE - N_IMGS
    pad = np.full((n_pad, H, H), PAD_BETA, np.float32)
    imgs_p = np.concatenate([imgs, pad], axis=0)
    shards = imgs_p.reshape(N_CORES, IMGS_PER_CORE, H, H)

    consts = np.ascontiguousarray(
        np.concatenate([A, -C], axis=1).astype(_bf16)
    )
    in_maps = []
    for c in range(N_CORES):
        sh = np.ascontiguousarray(
            shards[c].transpose(1, 0, 2).reshape(H, FD).astype(_bf16)
        )  # [128, 1408] bf16
        in_maps.append({"beta": sh, "consts": consts})

    nc = _build_bass()
    res = run_bass_kernel_spmd(
        nc,
        in_maps,
        core_ids=list(range(N_CORES)),
        trace_cores=list(range(N_CORES)) if os.environ.get("BASS_TRACE") else None,
    )

    outs = []
    for c in range(N_CORES):
        r = res.results[c]
        o = (
            r["out"]
            .astype(np.float32)
            .reshape(H, IMGS_PER_CORE, H)
            .transpose(1, 0, 2)
        )  # [11,H,H]
        outs.append(o)
    out_imgs = np.concatenate(outs, axis=0)[:N_IMGS]
    return out_imgs, res


def _host_gaps(beta_imgs, out_imgs, A, C):
    """gap0 and gap1 of the reference loop, from the device output.

    v1 = -out - beta;  gap0 = -sum(g0*v1);  gap1 = sum(g1*(v1 - s1)).
    """
    g0 = 1.0 / (1.0 + np.exp(-beta_imgs))
    v1 = -out_imgs - beta_imgs
    gap0 = -np.sum(g0 * v1, dtype=np.float64)
    g1 = (1.0 / (1.0 + np.exp(out_imgs))).astype(np.float32)  # sigmoid(v1+beta)
    s1 = -ALPHA * _energy_pool_np(g1, A, C)
    gap1 = float(np.sum(g1 * (v1 - s1), dtype=np.float64))
    return float(gap0), gap1, v1


def kernel(beta):
    beta = np.asarray(beta, dtype=np.float32)
    assert beta.shape == (B, C_CH, H, H), beta.shape

    out_imgs, _res = _run_device(beta)

    A, C = _build_mats()
    beta_i = beta.reshape(N_IMGS, H, H)
    gap0, gap1, v1 = _host_gaps(beta_i, out_imgs, A, C)

    if gap0 < TOL:
        # done latched before the first update: v stays 0
        return (-beta).astype(np.float32)

    if gap1 >= TOL:
        # loop did not freeze at t=1 -- exact numpy continuation from v1
        v = _fallback_loop(beta_i, v1, A, C, t_start=1, done=False)
        return (-(beta_i + v)).reshape(B, C_CH, H, H).astype(np.float32)

    return out_imgs.reshape(B, C_CH, H, H).astype(np.float32)
